# revision 17
# baseline (speedup 1.0000x reference)
"""Trainium2 Bass kernel for a single attention head (B=4, S=2048, D=4096, DH=128).

Sharding: 8 cores = (batch b, parity h). Core (b, h) owns q-tiles {h, h+2, ...,
h+14} of its batch -- even/odd striping balances the causal triangle exactly
(2(i+1) key-chunks for the i-th q-tile, i=0..7).

Host permutes keys per core to [own tiles | peer tiles] (each increasing), so
the SPMD graph is identical across cores:
  - pass P: K/V projections for the peer half (4 PSUM banks) while x streams,
  - pass M: K/V/Q for the own half (6 banks). x is read exactly once, split
    across the two HWDGE queues (sync/scalar).
  - attention per q-tile i: own chunks 0..i + peer chunks 0..i. The only
    mask-dependent blocks are the diagonal own chunk (constant triu block T)
    and the last peer chunk (all -1e9 for h=0, zeros for h=1, constant C);
    everything else computed is fully visible. T/C live in small constant
    SBUF tiles, sliced right-aligned -- no bulk mask DMA.

Attention keeps key chunks INTERLEAVED (own j -> slot 2j, peer j -> slot 2j+1)
in lm/w/v_sb so a tile's computed extent is always slots 0..2i+1: one DMA
transpose per tile (alternating queues) fills a pair-shared [128, slot, 256]
buffer and PV runs 256-wide over q-tile pairs (output transposed). The V bias
enters linearly, so it is dropped on device and added on the host; the output
is the unnormalized PV^T plus softmax row sums, divided on the host.
"""

import numpy as np
import ml_dtypes

import concourse.bass as bass
import concourse.tile as tile
from concourse import bacc, mybir
from concourse.bass_utils import run_bass_kernel_spmd

B, S, D, DH = 4, 2048, 4096, 128
SQ = S // 2          # q rows per core
N_CORES = 8
D_CH = D // 128      # 32 contraction chunks
QT = 8               # q row tiles per core

BF16 = mybir.dt.bfloat16
F32 = mybir.dt.float32


def build_nc():
    nc = bacc.Bacc(None)

    xT = nc.dram_tensor("xT", [D, S], BF16, kind="ExternalInput")
    # weights pre-tiled on host: w[p, i, m] = W[m, i*128+p]
    wqT = nc.dram_tensor("wqT", [128, D_CH, DH], BF16, kind="ExternalInput")
    wkT = nc.dram_tensor("wkT", [128, D_CH, DH], BF16, kind="ExternalInput")
    wvT = nc.dram_tensor("wvT", [128, D_CH, DH], BF16, kind="ExternalInput")
    bq = nc.dram_tensor("bq", [DH, 1], F32, kind="ExternalInput")
    bk = nc.dram_tensor("bk", [DH, 1], F32, kind="ExternalInput")
    maskT = nc.dram_tensor("maskT", [128, 128], BF16, kind="ExternalInput")
    maskC = nc.dram_tensor("maskC", [128, 128], BF16, kind="ExternalInput")
    outT = nc.dram_tensor("outT", [DH, SQ], BF16, kind="ExternalOutput")
    sums = nc.dram_tensor("sums", [128, QT], F32, kind="ExternalOutput")

    with tile.TileContext(nc) as tc:
        with (
            tc.tile_pool(name="weights", bufs=1) as wpool,
            tc.tile_pool(name="persist", bufs=1) as persist,
        ):
            w_sb = {}
            for name in ("q", "k", "v"):
                w_sb[name] = wpool.tile([128, D_CH, DH], BF16, tag=f"w{name}",
                                        name=f"w{name}")
            # k/v first: pass P needs them from chunk 0; q before pass M.
            # All on the gpsimd SWDGE queue so sync+scalar stream x tiles.
            for sl in range(4):
                for name, ext in (("k", wkT), ("v", wvT), ("q", wqT)):
                    ss = np.s_[:, sl * 8:(sl + 1) * 8, :]
                    nc.gpsimd.dma_start(out=w_sb[name][ss], in_=ext[ss])
            b_sb = {}
            for name, ext in (("k", bk), ("q", bq)):
                t = wpool.tile([DH, 1], F32, tag=f"b{name}")
                nc.gpsimd.dma_start(out=t[:], in_=ext[:])
                b_sb[name] = t

            kt_sb = persist.tile([DH, S], BF16, tag="kt")    # K^T [own|peer]
            vt_sb = persist.tile([DH, S], BF16, tag="vt")    # V^T [own|peer]
            qt_sb = persist.tile([DH, SQ], BF16, tag="qt")   # Q^T (own)
            # V chunks, slot-interleaved: own j -> 2j, peer j -> 2j+1
            v_sb = persist.tile([128, 16, DH], BF16, tag="v")
            sums_sb = persist.tile([128, QT], F32, tag="sums")

            # constant mask tiles as [128, 8, 128]: all-zero blocks except the
            # last; sliced right-aligned so the block hits the diagonal chunk
            zt_sb = persist.tile([128, QT, 128], BF16, tag="zt")
            zc_sb = persist.tile([128, QT, 128], BF16, tag="zc")
            nc.gpsimd.memset(zt_sb[:, :QT - 1, :], 0.0)
            nc.gpsimd.memset(zc_sb[:, :QT - 1, :], 0.0)
            nc.gpsimd.dma_start(out=zt_sb[:, QT - 1, :], in_=maskT[:])
            nc.gpsimd.dma_start(out=zc_sb[:, QT - 1, :], in_=maskC[:])

            # V-own accumulators live in their own 2 banks until attention
            # starts (the other 6 banks cycle P -> M(Q,K) -> logits/PV)
            with tc.tile_pool(name="psum_v", bufs=1, space="PSUM") as ppv:
                mv_acc = tuple(ppv.tile([DH, 512], F32, tag=f"mv{j}", name=f"mv{j}")
                               for j in range(2))

                # --- pass P: K/V for the peer half (x cols 1024:2048) ---
                with tc.tile_pool(name="psum_p", bufs=1, space="PSUM") as ppp:
                    acc = {}
                    for tag in ("pk0", "pk1", "pv0", "pv1"):
                        acc[tag] = ppp.tile([DH, 512], F32, tag=tag, name=tag)
                    with tc.tile_pool(name="xin_p", bufs=6) as xpp:
                        for i in range(D_CH):
                            xt = xpp.tile([128, SQ], BF16, tag="xt")
                            q = nc.sync if i % 2 == 0 else nc.scalar
                            q.dma_start(out=xt[:], in_=xT[i * 128:(i + 1) * 128, SQ:])
                            st = dict(start=(i == 0), stop=(i == D_CH - 1))
                            for j in range(2):
                                nc.tensor.matmul(acc[f"pk{j}"][:],
                                                 lhsT=w_sb["k"][:, i, :],
                                                 rhs=xt[:, j * 512:(j + 1) * 512], **st)
                            for j in range(2):
                                nc.tensor.matmul(acc[f"pv{j}"][:],
                                                 lhsT=w_sb["v"][:, i, :],
                                                 rhs=xt[:, j * 512:(j + 1) * 512], **st)
                    for j in range(2):
                        sl = np.s_[:, SQ + j * 512:SQ + (j + 1) * 512]
                        nc.vector.tensor_scalar_add(kt_sb[sl], acc[f"pk{j}"][:],
                                                    b_sb["k"][:])
                        nc.vector.tensor_copy(vt_sb[sl], acc[f"pv{j}"][:])
                    # peer-half V chunks -> odd slots (overlaps pass M)
                    nc.sync.dma_start_transpose(out=v_sb[:, 1:16:2, :],
                                                in_=vt_sb[:, SQ:])

                # --- pass M: K/V/Q for the own half (x cols 0:1024) ---
                with tc.tile_pool(name="psum_m", bufs=1, space="PSUM") as ppm:
                    acc = {}
                    for tag in ("mq0", "mq1", "mk0", "mk1"):
                        acc[tag] = ppm.tile([DH, 512], F32, tag=tag, name=tag)
                    with tc.tile_pool(name="xin_m", bufs=6) as xpm:
                        for i in range(D_CH):
                            xt = xpm.tile([128, SQ], BF16, tag="xt")
                            q = nc.sync if i % 2 == 0 else nc.scalar
                            q.dma_start(out=xt[:], in_=xT[i * 128:(i + 1) * 128, :SQ])
                            st = dict(start=(i == 0), stop=(i == D_CH - 1))
                            for j in range(2):
                                nc.tensor.matmul(acc[f"mk{j}"][:],
                                                 lhsT=w_sb["k"][:, i, :],
                                                 rhs=xt[:, j * 512:(j + 1) * 512], **st)
                            for j in range(2):
                                nc.tensor.matmul(mv_acc[j][:],
                                                 lhsT=w_sb["v"][:, i, :],
                                                 rhs=xt[:, j * 512:(j + 1) * 512], **st)
                            for j in range(2):
                                nc.tensor.matmul(acc[f"mq{j}"][:],
                                                 lhsT=w_sb["q"][:, i, :],
                                                 rhs=xt[:, j * 512:(j + 1) * 512], **st)
                    # Q and K evacuate first (they gate the first logits); V
                    # follows inside the attention block
                    for j in range(2):
                        sl = np.s_[:, j * 512:(j + 1) * 512]
                        nc.vector.tensor_scalar_add(qt_sb[sl], acc[f"mq{j}"][:],
                                                    b_sb["q"][:])
                    for j in range(2):
                        sl = np.s_[:, j * 512:(j + 1) * 512]
                        nc.vector.tensor_scalar_add(kt_sb[sl], acc[f"mk{j}"][:],
                                                    b_sb["k"][:])

                # --- attention, software-pipelined over q-tiles ---
                with (
                    tc.tile_pool(name="lm_sb", bufs=2) as lmpool,
                    tc.tile_pool(name="w_sb2", bufs=2) as wepool,
                    tc.tile_pool(name="wt_sb", bufs=2) as wtpool,
                    tc.tile_pool(name="o_sb", bufs=2) as opool,
                    tc.tile_pool(name="stats", bufs=8) as stat,
                    tc.tile_pool(name="l_psum", bufs=1, space="PSUM") as lpool,
                    tc.tile_pool(name="o_psum", bufs=2, space="PSUM") as popool,
                ):
                    pl_own = lpool.tile([128, QT, 128], F32, tag="pl_own",
                                        name="pl_own")
                    pl_peer = lpool.tile([128, QT, 128], F32, tag="pl_peer",
                                         name="pl_peer")
                    pair_bufs = {}

                    def soft(i):
                        e = (i + 1) * 128
                        qsl = np.s_[:, i * 128:(i + 1) * 128]
                        p = i // 2
                        if i % 2 == 0:
                            wt = wtpool.tile([128, 16, 256], BF16, tag="wt")
                            pair_bufs[p] = wt
                            # slots the even tile doesn't cover (its half only)
                            nc.gpsimd.memset(wt[:, 2 * i + 2:2 * i + 4, 0:128], 0.0)
                        else:
                            wt = pair_bufs[p]
                        half = np.s_[(i % 2) * 128:(i % 2) * 128 + 128]

                        for lo in range(0, e, 512):
                            w = min(512, e - lo)
                            nc.tensor.matmul(pl_own[:, lo // 128:(lo + w) // 128, :],
                                             lhsT=qt_sb[qsl],
                                             rhs=kt_sb[:, lo:lo + w],
                                             start=True, stop=True)
                        for lo in range(0, e, 512):
                            w = min(512, e - lo)
                            nc.tensor.matmul(pl_peer[:, lo // 128:(lo + w) // 128, :],
                                             lhsT=qt_sb[qsl],
                                             rhs=kt_sb[:, SQ + lo:SQ + lo + w],
                                             start=True, stop=True)

                        # lm[128, chunk, own/peer, 128] interleaves the halves
                        lm = lmpool.tile([128, QT, 2, 128], BF16, tag="lm")
                        nc.vector.tensor_add(lm[:, :i + 1, 0, :],
                                             pl_own[:, :i + 1, :],
                                             zt_sb[:, QT - 1 - i:, :])
                        nc.vector.tensor_add(lm[:, :i + 1, 1, :],
                                             pl_peer[:, :i + 1, :],
                                             zc_sb[:, QT - 1 - i:, :])
                        negmax = stat.tile([128, 1], F32, tag="negmax")
                        nc.vector.reduce_max(out=negmax[:], in_=lm[:, :i + 1, :, :],
                                             axis=mybir.AxisListType.XYZ, negate=True)

                        w_t = wepool.tile([128, QT, 2, 128], BF16, tag="w")
                        nc.scalar.activation(
                            out=w_t[:, :i + 1, :, :], in_=lm[:, :i + 1, :, :],
                            func=mybir.ActivationFunctionType.Exp,
                            bias=negmax[:], scale=1.0,
                            accum_out=sums_sb[:, i:i + 1])

                        tq = nc.sync if i % 2 == 0 else nc.scalar
                        tq.dma_start_transpose(out=wt[:, 0:2 * i + 2, half],
                                               in_=w_t[:, :i + 1, :, :])

                    def pv(p):
                        wt = pair_bufs.pop(p)
                        po = popool.tile([128, 256], F32, tag="poT")
                        ns = 4 * p + 4
                        for s in range(ns):
                            nc.tensor.matmul(po[:], lhsT=v_sb[:, s, :], rhs=wt[:, s, :],
                                             start=(s == 0), stop=(s == ns - 1))
                        o_sb = opool.tile([128, 256], BF16, tag="o")
                        nc.vector.tensor_copy(o_sb[:], po[:])
                        nc.gpsimd.dma_start(out=outT[:, p * 256:(p + 1) * 256],
                                            in_=o_sb[:])

                    for i in range(QT):
                        soft(i)
                        if i == 0:
                            # V-own evac + interleave into even slots; no bias
                            # (bv enters linearly; the host adds it)
                            for j in range(2):
                                sl = np.s_[:, j * 512:(j + 1) * 512]
                                nc.vector.tensor_copy(vt_sb[sl], mv_acc[j][:])
                            nc.sync.dma_start_transpose(out=v_sb[:, 0:16:2, :],
                                                        in_=vt_sb[:, :SQ])
                        if i in (3, 4, 5):
                            pv(i - 3)
                    pv(3)
                    nc.gpsimd.dma_start(out=sums[:], in_=sums_sb[:])

    nc.finalize()
    return nc


def shard_inputs(x, attn_mask, Wq, bq, Wk, bk, Wv, bv):
    """Host-side shard prep. Returns in_maps for cores 0..7."""
    bf = ml_dtypes.bfloat16
    xb = np.asarray(x).astype(bf)                   # cast first, like the reference
    mask_f = np.asarray(attn_mask)

    def tile_w(W):
        # [DH, D] -> [128, D_CH, DH] with w[p, i, m] = W[m, i*128+p]
        WT = np.asarray(W).astype(bf).T.reshape(D_CH, 128, DH)
        return np.ascontiguousarray(WT.transpose(1, 0, 2))

    wqt, wkt, wvt = tile_w(Wq), tile_w(Wk), tile_w(Wv)
    bqc = np.asarray(bq).astype(bf).astype(np.float32).reshape(DH, 1)
    bkc = np.asarray(bk).astype(bf).astype(np.float32).reshape(DH, 1)

    # constant mask blocks (causal structure: all diagonal blocks equal; all
    # first-superdiagonal blocks equal; all subdiagonal blocks equal)
    mT = np.ascontiguousarray(mask_f[0:128, 0:128].astype(bf))
    mC = {0: np.ascontiguousarray(mask_f[0:128, 128:256].astype(bf)),
          1: np.ascontiguousarray(mask_f[128:256, 0:128].astype(bf))}

    in_maps = []
    for c in range(N_CORES):
        b, h = divmod(c, 2)
        own = np.concatenate([np.arange(t * 128, (t + 1) * 128)
                              for t in range(h, 16, 2)])
        peer = np.concatenate([np.arange(t * 128, (t + 1) * 128)
                               for t in range(1 - h, 16, 2)])
        perm = np.concatenate([own, peer])
        xT = np.ascontiguousarray(xb[b][perm].T)                     # [D, S]
        in_maps.append({
            "xT": xT, "maskT": mT, "maskC": mC[h],
            "wqT": wqt, "wkT": wkt, "wvT": wvt,
            "bq": bqc, "bk": bkc,
        })
    return in_maps


def unshard(core_out, bv):
    """core_out: dicts with 'outT' [DH, SQ] bf16, 'sums' [128, QT] f32."""
    bvf = np.asarray(bv).astype(ml_dtypes.bfloat16).astype(np.float32)
    out = np.empty((B, S, DH), dtype=ml_dtypes.bfloat16)
    for c in range(N_CORES):
        b, h = divmod(c, 2)
        oT = np.asarray(core_out[c]["outT"], dtype=np.float32)
        sm = np.asarray(core_out[c]["sums"], dtype=np.float32)
        for j in range(QT):
            t = h + 2 * j
            blk = oT[:, j * 128:(j + 1) * 128] / sm[:, j][None, :]
            out[b, t * 128:(t + 1) * 128, :] = \
                (blk.T + bvf[None, :]).astype(ml_dtypes.bfloat16)
    return out


_NC_CACHE = {}


def kernel(x, attn_mask, Wq, bq, Wk, bk, Wv, bv):
    if "nc" not in _NC_CACHE:
        _NC_CACHE["nc"] = build_nc()
    nc = _NC_CACHE["nc"]
    in_maps = shard_inputs(x, attn_mask, Wq, bq, Wk, bk, Wv, bv)
    res = run_bass_kernel_spmd(nc, in_maps, list(range(N_CORES)))
    return unshard(res.results, bv)
